# revision 13
# baseline (speedup 1.0000x reference)
"""Bass/Trainium2 kernel for nn_CustomLoss_43834436223359 (retrieval_knn).

Strategy (v2):
  - Heavy part: brute-force KNN scan d2 = ||Tq - X||^2 over [B=256, N=200000]
    with top-50 per row.  X sharded row-wise across 8 cores (25000 cols each,
    padded to 13*2048 = 26624).
  - Per core, scores = sum_{d<127} Tq_d * X_d - 0.5*||x||^2 via PE matmuls
    (bias rides contraction row 127; ranking by score desc == d2 asc).
  - Selection is engine-balanced instead of a deep DVE tree:
      per 2048-col PSUM instance (4 matmuls, f32):
        ACT: one copy ps[:, 928:2048] -> bf16 staged tile     (~1.15us)
        DVE: one tensor_max(ps[:, 0:928], staged[0:928])      (~1.14us)
             -> bf16 "entries" (each covers 2 columns)
      The staged tail [928:1120) goes to HBM raw (1 column per value).
    No on-device top-k at all: entries + raw scores are DMA'd out and the
    host does argpartition + exact f64 rescoring (device values only gate
    candidate *coverage*, which has large margin).
  - DMA engine parallelism: X loads are issued from SP, entry/raw exports
    from the (otherwise idle) GPSIMD engine; transfers occupy the issuing
    engine's timeline, so they overlap compute on ACT/DVE/PE.
"""

import numpy as np
import ml_dtypes

BF16 = ml_dtypes.bfloat16

B, D, N, NQ, K = 256, 128, 200000, 10000, 50
NCORES = 8
SHARD = 25000
SEG = 2048                    # columns per instance (4 PSUM banks f32)
NSEG = 13
PADDED = NSEG * SEG           # 26624
W = 944                       # columns pair-merged by DVE per instance
WB = SEG - W                  # 1104 columns staged by ACT per instance
RAW = WB - W                  # 160 unpaired staged cols exported raw
NINST = 2 * NSEG              # 26 instances (2 query groups x 13 segments)
TAU = 0.1
EPS = 1e-8
ALPHA, BETA, LAMB, GAMMA = 1.0, 1.0, 1e-4, 1.0

# host-side selection breadth
T_ENT = 3100                  # top entries taken per row (each -> 2 cols)
T_RAW = 600                   # top raw scores taken per row (1 col each)

_cache = {}
last_results = None


def _patch_tail_drain():
    """Split the TileContext tail drain into one drain per pending proc:
    the stock implementation attaches a wait for EVERY proc in the global
    clock to a single Drain, overflowing the ISA's sync-wait slots."""
    import concourse.tile as tile
    from concourse.vector_clock import ScopedClock, VectorClock

    if getattr(tile.TileContext, "_ant_split_drain", False):
        return

    def _drain_and_barrier(self, tick_clock, wait_clock):
        vc = tick_clock.global_clock
        for proc in range(len(vc)):
            t = vc[proc]
            if t > 0:
                drain_inst = self.nc.sync.drain()
                sub = [0] * len(vc)
                sub[proc] = t
                wait_clock.add_sem_waits(
                    drain_inst.ins, ScopedClock({None: VectorClock(sub)})
                )
        self.nc.all_engine_barrier()
        assert self.sems is not None
        popped = self.nc._tile_sem_poison_stack.pop()
        assert popped is self._sem_poison
        self.nc.clear_and_free_semaphores(list(self.sems.allocated().values()))
        self.nc.all_engine_barrier()

    tile.TileContext._drain_and_barrier = _drain_and_barrier
    tile.TileContext._ant_split_drain = True


def _split_multi_waits(nc, max_waits=1):
    """Walrus legality pass: TRN2 instruction structs carry very few sync-wait
    slots (1 for Matmult/DMA/Activation/TensorTensor).  Hoist excess waits
    onto same-engine NoOps inserted right before the instruction — the engine
    queue stalls on the NoOp first, preserving semantics exactly."""
    import concourse.mybir as mybir
    f = nc.m.functions[0]
    for blk in f.blocks:
        insts = blk.instructions
        out = []
        changed = False
        for inst in insts:
            si = getattr(inst, "sync_info", None)
            if si is not None and len(si.on_wait) > max_waits:
                waits = list(si.on_wait)
                for w in waits[:-max_waits]:
                    nop = mybir.InstNoOp(name=f"I-wsplit-{nc.next_id()}")
                    nop.engine = inst.engine
                    nop.sync_info = mybir.SyncInfo(on_wait=[w], on_update=[])
                    out.append(nop)
                inst.sync_info = mybir.SyncInfo(
                    on_wait=waits[-max_waits:], on_update=list(si.on_update))
                changed = True
            out.append(inst)
        if changed:
            blk.instructions = out


def _build_bass(trace_sim=False):
    import concourse.bass as bass
    import concourse.mybir as mybir
    from concourse.tile import TileContext

    _patch_tail_drain()

    nc = bass.Bass()
    lhs_d = nc.dram_tensor("lhs", [128, 256], mybir.dt.bfloat16, kind="ExternalInput")
    xt_d = nc.dram_tensor("xt", [128, PADDED], mybir.dt.bfloat16, kind="ExternalInput")
    ent_d = nc.dram_tensor("ent", [128, NINST * W], mybir.dt.bfloat16,
                           kind="ExternalOutput")
    raw_d = nc.dram_tensor("raw", [128, NINST * RAW], mybir.dt.bfloat16,
                           kind="ExternalOutput")

    # matmul output splits (must not cross PSUM bank boundaries)
    A_MM = [(0, 512), (512, W)]
    B_MM = [(W, 1024), (1024, 1536), (1536, 2048)]

    with TileContext(nc, trace_sim=trace_sim) as tc:
        with (
            tc.tile_pool(name="xin", bufs=1) as xin_pool,
            tc.tile_pool(name="ps", bufs=1, space="PSUM") as psum_pool,
            tc.tile_pool(name="misc", bufs=1) as misc_pool,
        ):
            lhs_sb = misc_pool.tile([128, 256], mybir.dt.bfloat16, tag="lhs")
            nc.sync.dma_start(out=lhs_sb[:], in_=lhs_d[:])
            xt_tiles = [xin_pool.tile([128, SEG], mybir.dt.bfloat16,
                                      name=f"xt{s}", tag=f"xt{s}")
                        for s in range(NSEG)]
            # xt0 lands as two DMAs, B-region first, so the first instance's
            # B matmuls (and thus ACT) start ~1.6us earlier.
            nc.sync.dma_start(out=xt_tiles[0][:, W:SEG], in_=xt_d[:, W:SEG])
            nc.sync.dma_start(out=xt_tiles[0][:, 0:W], in_=xt_d[:, 0:W])
            for s in range(1, NSEG):
                nc.sync.dma_start(out=xt_tiles[s][:],
                                  in_=xt_d[:, s * SEG:(s + 1) * SEG])
            # Per parity, psA is read only by DVE (tensor_max in0), psB only
            # by ACT (staging copy).  The WAR chains then decouple per
            # engine: matmuls into psB wait on ACT alone, matmuls into psA
            # on DVE alone, so ACT/DVE never round-robin behind each other.
            # Sizes 944/1104 balance DVE (1.042 ns/elem on PSUM) against ACT
            # (0.833 ns/elem); the 160 staged-but-unpaired cols go out raw.
            ps_tiles = [psum_pool.tile([128, SEG], mybir.dt.float32,
                                       name=f"ps{p}", tag=f"ps{p}")
                        for p in range(2)]
            stg_all = misc_pool.tile([128, NINST * WB], mybir.dt.bfloat16, tag="stg")
            ent_all = misc_pool.tile([128, NINST * W], mybir.dt.bfloat16, tag="ent")

            # Warmup off the critical path while xt0 streams in: ramp the PE
            # p-state with junk matmuls and pre-load ACT's Copy table.
            wtile = misc_pool.tile([128, 512], mybir.dt.bfloat16, tag="wt")
            wjunk = misc_pool.tile([128, 16], mybir.dt.bfloat16, tag="wj")
            nc.vector.memset(wtile[:], 0.0)
            nc.scalar.copy(wjunk[:], wtile[:, 0:16])
            for _ in range(7):
                nc.tensor.matmul(ps_tiles[0][:, 0:512], wtile[:, 0:128],
                                 wtile[:], start=True, stop=True)

            for i in range(NINST):
                s, g = i // 2, i % 2
                xt_sb = xt_tiles[s]
                ps = ps_tiles[g]
                # B matmuls first so ACT can start while A matmuls run
                for lo, hi in B_MM + A_MM:
                    nc.tensor.matmul(
                        ps[:, lo:hi],
                        lhs_sb[:, g * 128:(g + 1) * 128],
                        xt_sb[:, lo:hi], start=True, stop=True)
                o = i * WB
                nc.scalar.copy(stg_all[:, o:o + WB], ps[:, W:SEG])
                nc.vector.tensor_max(ent_all[:, i * W:(i + 1) * W],
                                     ps[:, 0:W], stg_all[:, o:o + W])
                # export entries: per pair, but singly for the last two
                # instances to shorten the tail
                if i >= NINST - 2:
                    nc.gpsimd.dma_start(
                        out=ent_d[:, i * W:(i + 1) * W],
                        in_=ent_all[:, i * W:(i + 1) * W])
                elif g == 1:
                    nc.gpsimd.dma_start(
                        out=ent_d[:, (i - 1) * W:(i + 1) * W],
                        in_=ent_all[:, (i - 1) * W:(i + 1) * W])
                # export the raw tails of the staged tiles every 4 instances
                if i % 4 == 3 or i == NINST - 1:
                    i0 = (i // 4) * 4
                    kk = i - i0 + 1
                    nc.gpsimd.dma_start(
                        out=raw_d[:, i0 * RAW:(i + 1) * RAW].rearrange(
                            "p (k r) -> p k r", k=kk),
                        in_=stg_all[:, i0 * WB:(i + 1) * WB].rearrange(
                            "p (k t) -> p k t", k=kk)[:, :, W:WB])
    _split_multi_waits(nc)
    return nc


def _device_scores(Tq32, X32, xsq64):
    """Run the 8-core SPMD kernel; return per-core (ent, raw) bf16 arrays."""
    global last_results
    from concourse.bass_utils import run_bass_kernel_spmd

    if "nc" not in _cache:
        _cache["nc"] = _build_bass()
    nc = _cache["nc"]

    lhs = np.zeros([128, 256], np.float32)
    lhs[:127, :] = Tq32.T[:127, :]
    lhs[127, :] = 1.0
    lhs = lhs.astype(BF16)

    in_maps = []
    for c in range(NCORES):
        xt = np.zeros([128, PADDED], np.float32)
        sl = X32[c * SHARD:(c + 1) * SHARD]
        xt[:127, :SHARD] = sl.T[:127, :]
        xt[127, :SHARD] = (-0.5 * xsq64[c * SHARD:(c + 1) * SHARD]).astype(np.float32)
        xt[127, SHARD:] = -1e30
        in_maps.append({"lhs": lhs, "xt": xt.astype(BF16)})

    import time
    t0 = time.perf_counter()
    last_results = run_bass_kernel_spmd(nc, in_maps, core_ids=list(range(NCORES)))
    _cache["spmd_wall_s"] = time.perf_counter() - t0
    results = last_results.results

    ents = [np.asarray(r["ent"]) for r in results]   # [128, NINST*W] bf16
    raws = [np.asarray(r["raw"]) for r in results]   # [128, NINST*RAW] bf16
    return ents, raws


def _topk_exact(Tq64, X64, ents, raws, k=K):
    """Exact top-k per row from device candidate scores.  Entry (s, j)
    covers core-local cols {s*SEG + j, s*SEG + W + j}; raw (s, r) covers
    col s*SEG + 2W + r."""
    s = np.arange(NSEG)
    j = np.arange(W)
    e0 = (s[:, None] * SEG + j[None, :]).ravel()          # [NSEG*W]
    e1 = e0 + W
    r0 = (s[:, None] * SEG + 2 * W + np.arange(RAW)[None, :]).ravel()
    # per-core global column offsets; pad locals (>= SHARD) masked per map
    ecols0 = np.concatenate([c * SHARD + e0 for c in range(NCORES)])
    ecols1 = np.concatenate([c * SHARD + e1 for c in range(NCORES)])
    rcols = np.concatenate([c * SHARD + r0 for c in range(NCORES)])
    e0valid = np.tile(e0 < SHARD, NCORES)
    e1valid = np.tile(e1 < SHARD, NCORES)
    rvalid = np.tile(r0 < SHARD, NCORES)
    evalid = e0valid | e1valid

    out = np.empty((B, k), np.int64)
    for q in range(B):
        g, p = divmod(q, 128)
        ev = np.concatenate(
            [e[p].reshape(NINST, W)[g::2].ravel() for e in ents]).astype(np.float32)
        rv = np.concatenate(
            [r[p].reshape(NINST, RAW)[g::2].ravel() for r in raws]).astype(np.float32)
        ev[~evalid] = -np.inf
        rv[~rvalid] = -np.inf
        te = np.argpartition(-ev, T_ENT)[:T_ENT]
        tr = np.argpartition(-rv, T_RAW)[:T_RAW]
        cc = np.concatenate([ecols0[te][e0valid[te]],
                             ecols1[te][e1valid[te]],
                             rcols[tr][rvalid[tr]]])
        cc = np.unique(cc)
        diff = X64[cc] - Tq64[q]
        d2 = np.einsum("ij,ij->i", diff, diff)
        order = np.lexsort((cc, d2))
        out[q] = cc[order[:k]]
    return out


def _sqdist(A, Bm):
    d2 = (A * A).sum(1)[:, None] + (Bm * Bm).sum(1)[None, :] - 2.0 * (A @ Bm.T)
    return np.maximum(d2, 0.0)


def _host_loss(q_batch, X, Wm, b, pre_weights, pre_indices, q_indices, idx, post_idx):
    """Mirror of reference() in numpy f64, given the KNN indices."""
    Tq = q_batch @ Wm.T + b
    s, t = Tq, X[idx]
    comb = np.concatenate([s, t], 0)
    sigma_sq = np.median(_sqdist(comb, comb)) / 2.0
    if sigma_sq < 1e-6:
        sigma_sq = 1.0
    g = 1.0 / (sigma_sq + EPS)
    kxx = np.exp(-g * _sqdist(s, s)).mean()
    kyy = np.exp(-g * _sqdist(t, t)).mean()
    kxy = np.exp(-g * _sqdist(s, t)).mean()
    loss_dist = max(kxx + kyy - 2.0 * kxy, 0.0)
    Xn = X[post_idx]                                   # [B, K, d]
    l2 = ((Tq[:, None, :] - Xn) ** 2).sum(-1)          # [B, K]
    z = -l2 / TAU
    z = z - z.max(1, keepdims=True)
    ez = np.exp(z)
    post_w = ez / ez.sum(1, keepdims=True)
    pre_i = pre_indices[q_indices]                     # [B, K]
    pre_w = pre_weights[q_indices]                     # [B, K]
    cat = np.concatenate([pre_i, post_idx], axis=1)    # [B, 2K]
    mult = (cat[:, :, None] == cat[:, None, :]).sum(-1).astype(np.float64)
    p_raw = np.einsum("bmk,bk->bm",
                      (cat[:, :, None] == pre_i[:, None, :]).astype(np.float64), pre_w)
    q_raw = np.einsum("bmk,bk->bm",
                      (cat[:, :, None] == post_idx[:, None, :]).astype(np.float64), post_w)
    p_c = np.maximum(p_raw, EPS)
    q_c = np.maximum(q_raw, EPS)
    p = p_c / (p_c / mult).sum(1, keepdims=True)
    q = q_c / (q_c / mult).sum(1, keepdims=True)
    kl = ((p * (np.log(p) - np.log(q))) / mult).sum(1)
    loss_knn = kl.mean()
    loss_reg = 0.5 * ((Wm ** 2).sum() + (b ** 2).sum())
    loss_anchor = ((Tq - q_batch) ** 2).sum(1).mean()
    total = ALPHA * loss_dist + BETA * loss_knn + LAMB * loss_reg + GAMMA * loss_anchor
    return np.stack([total, loss_dist, loss_knn, loss_anchor]).astype(np.float32)


def kernel(q_batch, X, W, b, pre_weights, pre_indices, q_indices, idx):
    q_batch = np.asarray(q_batch, np.float32)
    X32 = np.asarray(X, np.float32)
    W32 = np.asarray(W, np.float32)
    b32 = np.asarray(b, np.float32)
    pre_weights = np.asarray(pre_weights, np.float64)
    pre_indices = np.asarray(pre_indices, np.int64)
    q_indices = np.asarray(q_indices, np.int64)
    idx = np.asarray(idx, np.int64)

    Tq32 = q_batch @ W32.T + b32
    X64 = X32.astype(np.float64)
    Tq64 = Tq32.astype(np.float64)
    xsq64 = (X64 * X64).sum(1)

    ents, raws = _device_scores(Tq32, X32, xsq64)
    post_idx = _topk_exact(Tq64, X64, ents, raws)

    return _host_loss(q_batch.astype(np.float64), X64, W32.astype(np.float64),
                      b32.astype(np.float64), pre_weights, pre_indices,
                      q_indices, idx, post_idx)


# revision 14
# speedup vs baseline: 1.3099x; 1.3099x over previous
"""Bass/Trainium2 kernel for nn_CustomLoss_43834436223359 (retrieval_knn).

Strategy (v2):
  - Heavy part: brute-force KNN scan d2 = ||Tq - X||^2 over [B=256, N=200000]
    with top-50 per row.  X sharded row-wise across 8 cores (25000 cols each,
    padded to 13*2048 = 26624).
  - Per core, scores = sum_{d<127} Tq_d * X_d - 0.5*||x||^2 via PE matmuls
    (bias rides contraction row 127; ranking by score desc == d2 asc).
  - Selection is engine-balanced instead of a deep DVE tree:
      per 2048-col PSUM instance (4 matmuls, f32):
        ACT: one copy ps[:, 928:2048] -> bf16 staged tile     (~1.15us)
        DVE: one tensor_max(ps[:, 0:928], staged[0:928])      (~1.14us)
             -> bf16 "entries" (each covers 2 columns)
      The staged tail [928:1120) goes to HBM raw (1 column per value).
    No on-device top-k at all: entries + raw scores are DMA'd out and the
    host does argpartition + exact f64 rescoring (device values only gate
    candidate *coverage*, which has large margin).
  - DMA engine parallelism: X loads are issued from SP, entry/raw exports
    from the (otherwise idle) GPSIMD engine; transfers occupy the issuing
    engine's timeline, so they overlap compute on ACT/DVE/PE.
"""

import numpy as np
import ml_dtypes

BF16 = ml_dtypes.bfloat16

B, D, N, NQ, K = 256, 128, 200000, 10000, 50
NCORES = 8
SHARD = 25000
SEG = 2048                    # columns per instance (4 PSUM banks f32)
NSEG = 13
PADDED = NSEG * SEG           # 26624
W = 1024                      # columns pair-merged by DVE per instance
WB = SEG - W                  # 1024 columns staged by ACT per instance
NINST = 2 * NSEG              # 26 instances (2 query groups x 13 segments)
TAU = 0.1
EPS = 1e-8
ALPHA, BETA, LAMB, GAMMA = 1.0, 1.0, 1e-4, 1.0

# host-side selection breadth
T_ENT = 3400                  # top entries taken per row (each -> 2 cols)

_cache = {}
last_results = None


def _patch_tail_drain():
    """Split the TileContext tail drain into one drain per pending proc:
    the stock implementation attaches a wait for EVERY proc in the global
    clock to a single Drain, overflowing the ISA's sync-wait slots."""
    import concourse.tile as tile
    from concourse.vector_clock import ScopedClock, VectorClock

    if getattr(tile.TileContext, "_ant_split_drain", False):
        return

    def _drain_and_barrier(self, tick_clock, wait_clock):
        vc = tick_clock.global_clock
        for proc in range(len(vc)):
            t = vc[proc]
            if t > 0:
                drain_inst = self.nc.sync.drain()
                sub = [0] * len(vc)
                sub[proc] = t
                wait_clock.add_sem_waits(
                    drain_inst.ins, ScopedClock({None: VectorClock(sub)})
                )
        self.nc.all_engine_barrier()
        assert self.sems is not None
        popped = self.nc._tile_sem_poison_stack.pop()
        assert popped is self._sem_poison
        self.nc.clear_and_free_semaphores(list(self.sems.allocated().values()))
        self.nc.all_engine_barrier()

    tile.TileContext._drain_and_barrier = _drain_and_barrier
    tile.TileContext._ant_split_drain = True


def _split_multi_waits(nc, max_waits=1):
    """Walrus legality pass: TRN2 instruction structs carry very few sync-wait
    slots (1 for Matmult/DMA/Activation/TensorTensor).  Hoist excess waits
    onto same-engine NoOps inserted right before the instruction — the engine
    queue stalls on the NoOp first, preserving semantics exactly."""
    import concourse.mybir as mybir
    f = nc.m.functions[0]
    for blk in f.blocks:
        insts = blk.instructions
        out = []
        changed = False
        for inst in insts:
            si = getattr(inst, "sync_info", None)
            if si is not None and len(si.on_wait) > max_waits:
                waits = list(si.on_wait)
                for w in waits[:-max_waits]:
                    nop = mybir.InstNoOp(name=f"I-wsplit-{nc.next_id()}")
                    nop.engine = inst.engine
                    nop.sync_info = mybir.SyncInfo(on_wait=[w], on_update=[])
                    out.append(nop)
                inst.sync_info = mybir.SyncInfo(
                    on_wait=waits[-max_waits:], on_update=list(si.on_update))
                changed = True
            out.append(inst)
        if changed:
            blk.instructions = out


def _build_bass(trace_sim=False):
    import concourse.bass as bass
    import concourse.mybir as mybir
    from concourse.tile import TileContext

    _patch_tail_drain()

    nc = bass.Bass()
    lhs_d = nc.dram_tensor("lhs", [128, 256], mybir.dt.bfloat16, kind="ExternalInput")
    xt_d = nc.dram_tensor("xt", [128, PADDED], mybir.dt.bfloat16, kind="ExternalInput")
    ent_d = nc.dram_tensor("ent", [128, NINST * W], mybir.dt.bfloat16,
                           kind="ExternalOutput")

    with TileContext(nc, trace_sim=trace_sim) as tc:
        with (
            tc.tile_pool(name="xin", bufs=1) as xin_pool,
            tc.tile_pool(name="ps", bufs=1, space="PSUM") as psum_pool,
            tc.tile_pool(name="misc", bufs=1) as misc_pool,
        ):
            lhs_sb = misc_pool.tile([128, 256], mybir.dt.bfloat16, tag="lhs")
            nc.sync.dma_start(out=lhs_sb[:], in_=lhs_d[:])
            xt_tiles = [xin_pool.tile([128, SEG], mybir.dt.bfloat16,
                                      name=f"xt{s}", tag=f"xt{s}")
                        for s in range(NSEG)]
            # xt0 lands as two DMAs, B-region first, so the first instance's
            # B matmuls (and thus ACT) start ~1.6us earlier.
            nc.sync.dma_start(out=xt_tiles[0][:, W:SEG], in_=xt_d[:, W:SEG])
            nc.sync.dma_start(out=xt_tiles[0][:, 0:W], in_=xt_d[:, 0:W])
            for s in range(1, NSEG):
                nc.sync.dma_start(out=xt_tiles[s][:],
                                  in_=xt_d[:, s * SEG:(s + 1) * SEG])
            # Per parity, psA is read only by DVE (tensor_max in0), psB only
            # by ACT (staging copy).  The WAR chains then decouple per
            # engine: matmuls into psB wait on ACT alone, matmuls into psA
            # on DVE alone, so ACT/DVE never round-robin behind each other.
            # Sizes 944/1104 balance DVE (1.042 ns/elem on PSUM) against ACT
            # (0.833 ns/elem); the 160 staged-but-unpaired cols go out raw.
            psA = [psum_pool.tile([128, W], mybir.dt.float32,
                                  name=f"psA{p}", tag=f"psA{p}") for p in range(2)]
            psB = [psum_pool.tile([128, WB], mybir.dt.float32,
                                  name=f"psB{p}", tag=f"psB{p}") for p in range(2)]
            stg_all = misc_pool.tile([128, NINST * WB], mybir.dt.bfloat16, tag="stg")
            ent_all = misc_pool.tile([128, NINST * W], mybir.dt.bfloat16, tag="ent")

            # Warmup off the critical path while xt0 streams in: ramp the PE
            # p-state with junk matmuls and pre-load ACT's Copy table.
            wtile = misc_pool.tile([128, 512], mybir.dt.bfloat16, tag="wt")
            wjunk = misc_pool.tile([128, 16], mybir.dt.bfloat16, tag="wj")
            nc.vector.memset(wtile[:], 0.0)
            nc.scalar.copy(wjunk[:], wtile[:, 0:16])
            for _ in range(7):
                nc.tensor.matmul(psA[0][:, 0:512], wtile[:, 0:128],
                                 wtile[:], start=True, stop=True)

            for i in range(NINST):
                s, g = i // 2, i % 2
                xt_sb = xt_tiles[s]
                # B matmuls first so ACT can start while A matmuls run
                for m in range(WB // 512):
                    nc.tensor.matmul(
                        psB[g][:, m * 512:(m + 1) * 512],
                        lhs_sb[:, g * 128:(g + 1) * 128],
                        xt_sb[:, W + m * 512:W + (m + 1) * 512],
                        start=True, stop=True)
                for m in range(W // 512):
                    nc.tensor.matmul(
                        psA[g][:, m * 512:(m + 1) * 512],
                        lhs_sb[:, g * 128:(g + 1) * 128],
                        xt_sb[:, m * 512:(m + 1) * 512],
                        start=True, stop=True)
                o = i * WB
                nc.scalar.copy(stg_all[:, o:o + WB], psB[g][:])
                nc.vector.tensor_max(ent_all[:, i * W:(i + 1) * W],
                                     psA[g][:], stg_all[:, o:o + W])
                # export entries: per pair, but singly for the last two
                # instances to shorten the tail
                if i >= NINST - 2:
                    nc.gpsimd.dma_start(
                        out=ent_d[:, i * W:(i + 1) * W],
                        in_=ent_all[:, i * W:(i + 1) * W])
                elif g == 1:
                    nc.gpsimd.dma_start(
                        out=ent_d[:, (i - 1) * W:(i + 1) * W],
                        in_=ent_all[:, (i - 1) * W:(i + 1) * W])
    _split_multi_waits(nc)
    return nc


def _device_scores(Tq32, X32, xsq64):
    """Run the 8-core SPMD kernel; return per-core (ent, raw) bf16 arrays."""
    global last_results
    from concourse.bass_utils import run_bass_kernel_spmd

    if "nc" not in _cache:
        _cache["nc"] = _build_bass()
    nc = _cache["nc"]

    lhs = np.zeros([128, 256], np.float32)
    lhs[:127, :] = Tq32.T[:127, :]
    lhs[127, :] = 1.0
    lhs = lhs.astype(BF16)

    in_maps = []
    for c in range(NCORES):
        xt = np.zeros([128, PADDED], np.float32)
        sl = X32[c * SHARD:(c + 1) * SHARD]
        xt[:127, :SHARD] = sl.T[:127, :]
        xt[127, :SHARD] = (-0.5 * xsq64[c * SHARD:(c + 1) * SHARD]).astype(np.float32)
        xt[127, SHARD:] = -1e30
        in_maps.append({"lhs": lhs, "xt": xt.astype(BF16)})

    import time
    t0 = time.perf_counter()
    last_results = run_bass_kernel_spmd(nc, in_maps, core_ids=list(range(NCORES)))
    _cache["spmd_wall_s"] = time.perf_counter() - t0
    results = last_results.results

    ents = [np.asarray(r["ent"]) for r in results]   # [128, NINST*W] bf16
    return ents


def _topk_exact(Tq64, X64, ents, k=K):
    """Exact top-k per row from device candidate scores.  Entry (s, j)
    covers core-local cols {s*SEG + j, s*SEG + W + j}."""
    s = np.arange(NSEG)
    j = np.arange(W)
    e0 = (s[:, None] * SEG + j[None, :]).ravel()          # [NSEG*W]
    e1 = e0 + W
    # per-core global column offsets; pad locals (>= SHARD) masked per map
    ecols0 = np.concatenate([c * SHARD + e0 for c in range(NCORES)])
    ecols1 = np.concatenate([c * SHARD + e1 for c in range(NCORES)])
    e0valid = np.tile(e0 < SHARD, NCORES)
    e1valid = np.tile(e1 < SHARD, NCORES)
    evalid = e0valid | e1valid

    out = np.empty((B, k), np.int64)
    for q in range(B):
        g, p = divmod(q, 128)
        ev = np.concatenate(
            [e[p].reshape(NINST, W)[g::2].ravel() for e in ents]).astype(np.float32)
        ev[~evalid] = -np.inf
        te = np.argpartition(-ev, T_ENT)[:T_ENT]
        cc = np.concatenate([ecols0[te][e0valid[te]],
                             ecols1[te][e1valid[te]]])
        cc = np.unique(cc)
        diff = X64[cc] - Tq64[q]
        d2 = np.einsum("ij,ij->i", diff, diff)
        order = np.lexsort((cc, d2))
        out[q] = cc[order[:k]]
    return out


def _sqdist(A, Bm):
    d2 = (A * A).sum(1)[:, None] + (Bm * Bm).sum(1)[None, :] - 2.0 * (A @ Bm.T)
    return np.maximum(d2, 0.0)


def _host_loss(q_batch, X, Wm, b, pre_weights, pre_indices, q_indices, idx, post_idx):
    """Mirror of reference() in numpy f64, given the KNN indices."""
    Tq = q_batch @ Wm.T + b
    s, t = Tq, X[idx]
    comb = np.concatenate([s, t], 0)
    sigma_sq = np.median(_sqdist(comb, comb)) / 2.0
    if sigma_sq < 1e-6:
        sigma_sq = 1.0
    g = 1.0 / (sigma_sq + EPS)
    kxx = np.exp(-g * _sqdist(s, s)).mean()
    kyy = np.exp(-g * _sqdist(t, t)).mean()
    kxy = np.exp(-g * _sqdist(s, t)).mean()
    loss_dist = max(kxx + kyy - 2.0 * kxy, 0.0)
    Xn = X[post_idx]                                   # [B, K, d]
    l2 = ((Tq[:, None, :] - Xn) ** 2).sum(-1)          # [B, K]
    z = -l2 / TAU
    z = z - z.max(1, keepdims=True)
    ez = np.exp(z)
    post_w = ez / ez.sum(1, keepdims=True)
    pre_i = pre_indices[q_indices]                     # [B, K]
    pre_w = pre_weights[q_indices]                     # [B, K]
    cat = np.concatenate([pre_i, post_idx], axis=1)    # [B, 2K]
    mult = (cat[:, :, None] == cat[:, None, :]).sum(-1).astype(np.float64)
    p_raw = np.einsum("bmk,bk->bm",
                      (cat[:, :, None] == pre_i[:, None, :]).astype(np.float64), pre_w)
    q_raw = np.einsum("bmk,bk->bm",
                      (cat[:, :, None] == post_idx[:, None, :]).astype(np.float64), post_w)
    p_c = np.maximum(p_raw, EPS)
    q_c = np.maximum(q_raw, EPS)
    p = p_c / (p_c / mult).sum(1, keepdims=True)
    q = q_c / (q_c / mult).sum(1, keepdims=True)
    kl = ((p * (np.log(p) - np.log(q))) / mult).sum(1)
    loss_knn = kl.mean()
    loss_reg = 0.5 * ((Wm ** 2).sum() + (b ** 2).sum())
    loss_anchor = ((Tq - q_batch) ** 2).sum(1).mean()
    total = ALPHA * loss_dist + BETA * loss_knn + LAMB * loss_reg + GAMMA * loss_anchor
    return np.stack([total, loss_dist, loss_knn, loss_anchor]).astype(np.float32)


def kernel(q_batch, X, W, b, pre_weights, pre_indices, q_indices, idx):
    q_batch = np.asarray(q_batch, np.float32)
    X32 = np.asarray(X, np.float32)
    W32 = np.asarray(W, np.float32)
    b32 = np.asarray(b, np.float32)
    pre_weights = np.asarray(pre_weights, np.float64)
    pre_indices = np.asarray(pre_indices, np.int64)
    q_indices = np.asarray(q_indices, np.int64)
    idx = np.asarray(idx, np.int64)

    Tq32 = q_batch @ W32.T + b32
    X64 = X32.astype(np.float64)
    Tq64 = Tq32.astype(np.float64)
    xsq64 = (X64 * X64).sum(1)

    ents = _device_scores(Tq32, X32, xsq64)
    post_idx = _topk_exact(Tq64, X64, ents)

    return _host_loss(q_batch.astype(np.float64), X64, W32.astype(np.float64),
                      b32.astype(np.float64), pre_weights, pre_indices,
                      q_indices, idx, post_idx)
